# revision 7
# baseline (speedup 1.0000x reference)
"""Trainium2 Bass kernel for nn_AbsoluteMinimalBlock (rmsnorm -> rank-1 SSM scan -> rmsnorm -> rank-2 FFN).

Math: the whole block is a rank-3 update of x:
    out[t,d] = x[t,d] + h[t]*Wout[d] + g0[t]*W20[d] + g1[t]*W21[d]
  driven by 5 per-token reductions over D:
    d1 = x@(nw*W_in), dW = x@Wout, dA = x@(nw*w1_0), dB = x@(nw*w1_1), S0 = sum(x^2)
  with rstd1 = 1/sqrt(S0/D+eps); u = d1*rstd1; h = scan(a, u);
  D*ms2 = S0 + 2h*dW + h^2*|Wout|^2 (analytic); p_r = (d_r + h*(Wout.W1r))*rstd2;
  g_r = gelu_tanh(p_r * 1).

Sharding: 8 cores = 4 batches x 2 sequence halves; each core prepends a
PRE-tile prefix (zeros for first half / tail of first half for second) so the
scan state is exact to fp32 (a^128 per tile of decay).

Device: token-major tiles [128 tok x 1024]; token t -> (tile t//128, partition
t%128). Reductions over D: PE transposes x into PSUM (8x 128x128 per tile,
fp32 transposes are fast), DVE copies to SBUF as float32r, then f32r matmuls
(lhsT = 4 weight columns per d-slice) accumulate dots [4 x 512] per 4-tile
group; small PE transposes flip them to the per-token [128 x NT] layout.
S0 rides ACT square's accum_out. The scan is 3 matmuls (triangular T128,
a^(127-k) finals row, K=1 carry broadcast) + one 1-partition
tensor_tensor_scan over tile finals. Rank-3 reconstruction: per-tile K=3 f32r
matmuls against W3 [3 x 1024]; DVE adds the PSUM update into the resident x
tile in place; DMA out.
"""
import sys, os
for _p in ("/root/.axon_site/_ro/trn_rl_repo", "/opt/trn_rl_repo"):
    if os.path.isdir(_p) and _p not in sys.path:
        sys.path.append(_p)

import numpy as np
import concourse.bass as bass
import concourse.bacc as bacc
import concourse.mybir as mybir
import concourse.tile as tile
from concourse.bass_utils import run_bass_kernel_spmd

F32 = mybir.dt.float32
F32R = mybir.dt.float32r
AF = mybir.ActivationFunctionType
OP = mybir.AluOpType

N_CORES = 8
B, S, D = 4, 8192, 1024
HALF = S // 2
MAIN_TILES = HALF // 128      # 32
EPS = 1e-6

_cache = {}


def build_program(nt: int, reps=None):
    """SPMD program for nt total tiles/core (nt-32 prefix tiles).
    reps: wrap body in For_i for timing probes (None = plain)."""
    pre_tiles = nt - MAIN_TILES
    groups = []
    i = 0
    while i < nt:
        groups.append(list(range(i, min(i + 4, nt))))
        i += 4

    nc = bacc.Bacc("TRN2", target_bir_lowering=False, debug=False, num_devices=N_CORES)

    xd = nc.dram_tensor("x_in", [nt * 128, D], F32, kind="ExternalInput").ap()
    yd = nc.dram_tensor("y_out", [HALF, D], F32, kind="ExternalOutput").ap()
    vw4d = nc.dram_tensor("vw4", [D, 4], F32, kind="ExternalInput").ap()
    w3d = nc.dram_tensor("w3", [3, D], F32, kind="ExternalInput").ap()
    identd = nc.dram_tensor("ident", [128, 128], F32, kind="ExternalInput").ap()
    t128d = nc.dram_tensor("t128", [128, 128], F32, kind="ExternalInput").ap()
    frowd = nc.dram_tensor("frow", [128, 1], F32, kind="ExternalInput").ap()
    apow1d = nc.dram_tensor("apow1", [1, 128], F32, kind="ExternalInput").ap()
    alrowd = nc.dram_tensor("alrow", [1, nt], F32, kind="ExternalInput").ap()
    colsd = nc.dram_tensor("cols3", [128, 3], F32, kind="ExternalInput").ap()

    with tile.TileContext(nc) as tc:
        with (
            tc.tile_pool(name="xpool", bufs=1) as xpool,
            tc.tile_pool(name="work", bufs=3) as work,
            tc.tile_pool(name="sq", bufs=2) as sqp,
            tc.tile_pool(name="small", bufs=1) as small,
            tc.tile_pool(name="cst", bufs=1) as cst,
            tc.tile_pool(name="ps", bufs=1, space="PSUM") as psp,
        ):
            # ---- constants ----
            vw4 = cst.tile([128, 8, 4], F32, name="vw4")
            vw4r = cst.tile([128, 8, 4], F32R, name="vw4r")
            w3r = cst.tile([3, D], F32R, name="w3r")
            ident = cst.tile([128, 128], F32, name="ident")
            t128 = cst.tile([128, 128], F32, name="t128")
            frow = cst.tile([128, 1], F32, name="frow")
            apow1 = cst.tile([1, 128], F32, name="apow1")
            alrow = cst.tile([1, nt], F32, name="alrow")
            cols3 = cst.tile([128, 3], F32, name="cols3")
            nc.sync.dma_start(vw4[:], vw4d.rearrange("(k p) q -> p k q", p=128))
            nc.vector.tensor_copy(vw4r[:], vw4[:])
            w3f = cst.tile([3, D], F32, name="w3f")
            nc.sync.dma_start(w3f[:], w3d[:])
            nc.vector.tensor_copy(w3r[:], w3f[:])
            nc.sync.dma_start(ident[:], identd[:])
            nc.sync.dma_start(t128[:], t128d[:])
            nc.sync.dma_start(frow[:], frowd[:])
            nc.sync.dma_start(apow1[:], apow1d[:])
            nc.sync.dma_start(alrow[:], alrowd[:])
            nc.sync.dma_start(cols3[:], colsd[:])

            # ---- per-token arrays ----
            d4 = small.tile([128, nt, 4], F32, name="d4")
            s0 = small.tile([128, nt], F32, name="s0")
            hg = small.tile([128, 3, nt], F32, name="hg")
            u = small.tile([128, nt], F32, name="u")
            scr1 = small.tile([128, nt], F32, name="scr1")
            scr2 = small.tile([128, nt], F32, name="scr2")
            rstd2 = small.tile([128, nt], F32, name="rstd2")
            srow = small.tile([1, nt], F32, name="srow")
            eps_col = small.tile([128, 1], F32, name="eps_col")
            nc.vector.memset(eps_col[:], float(EPS))
            crow = small.tile([1, nt], F32, name="crow")

            def body():
                x_tiles = [xpool.tile([128, D], F32, tag=f"x{i}", name=f"x{i}")
                           for i in range(nt)]
                # ---------------- phase A ----------------
                for g in groups:
                    glen = len(g)
                    for i in g:
                        nc.sync.dma_start(x_tiles[i][:], xd[i * 128:(i + 1) * 128, :])
                        sq = sqp.tile([128, D], F32, tag="sq", name=f"sq{i}")
                        nc.scalar.activation(sq[:], x_tiles[i][:], AF.Square,
                                             accum_out=s0[:, i:i + 1])
                    dots_ps = psp.tile([4, 512], F32, tag="dots_ps",
                                       name=f"dots_ps{g[0]}", bufs=2)
                    for p in range(4):          # slice pairs (2p, 2p+1)
                        pair_ps = psp.tile([128, D], F32, tag="big_ps",
                                           name=f"pair_ps{g[0]}_{p}", bufs=2)
                        for sl in range(2):
                            k = 2 * p + sl
                            for gi, i in enumerate(g):
                                nc.tensor.transpose(
                                    pair_ps[:, sl * 512 + gi * 128:
                                            sl * 512 + (gi + 1) * 128],
                                    x_tiles[i][:, k * 128:(k + 1) * 128],
                                    ident[:])
                        pair_sb = work.tile([128, D], F32R, tag="pair_sb",
                                            name=f"pair_sb{g[0]}_{p}")
                        if glen == 4:
                            nc.vector.tensor_copy(pair_sb[:], pair_ps[:])
                        else:
                            for sl in range(2):
                                nc.vector.tensor_copy(
                                    pair_sb[:, sl * 512:sl * 512 + glen * 128],
                                    pair_ps[:, sl * 512:sl * 512 + glen * 128])
                        for sl in range(2):
                            k = 2 * p + sl
                            nc.tensor.matmul(dots_ps[:, 0:glen * 128],
                                             vw4r[:, k, :],
                                             pair_sb[:, sl * 512:sl * 512 + glen * 128],
                                             start=(p == 0 and sl == 0),
                                             stop=(p == 3 and sl == 1))
                    dots_sb = work.tile([4, 512], F32, tag="dots_sb",
                                        name=f"dots_sb{g[0]}")
                    nc.vector.tensor_copy(dots_sb[:, 0:glen * 128],
                                          dots_ps[:, 0:glen * 128])
                    for gi, i in enumerate(g):
                        d4t_ps = psp.tile([128, 4], F32, tag="misc_ps",
                                          name=f"d4t_ps{i}", bufs=2)
                        nc.tensor.transpose(d4t_ps[:],
                                            dots_sb[:, gi * 128:(gi + 1) * 128],
                                            ident[0:4, 0:4])
                        nc.vector.tensor_copy(d4[:, i, :], d4t_ps[:])

                # ---------------- scan ----------------
                nc.scalar.activation(scr1[:], s0[:], AF.Sqrt,
                                     bias=eps_col[:], scale=float(1.0 / D))
                nc.vector.reciprocal(scr2[:], scr1[:])
                nc.vector.tensor_mul(u[:], d4[:, :, 0], scr2[:])

                loc_ps = psp.tile([128, nt], F32, tag="misc_ps", name="loc_ps", bufs=2)
                f_ps = psp.tile([1, nt], F32, tag="misc_ps", name="f_ps", bufs=2)
                nc.tensor.matmul(loc_ps[:], t128[:], u[:], start=True, stop=False)
                nc.tensor.matmul(f_ps[:], frow[:], u[:], start=True, stop=True)
                nc.vector.tensor_tensor_scan(srow[:], alrow[:], f_ps[:], 0.0,
                                             OP.mult, OP.add)
                nc.vector.memset(crow[0:1, 0:1], 0.0)
                nc.vector.tensor_copy(crow[0:1, 1:nt], srow[0:1, 0:nt - 1])
                nc.tensor.matmul(loc_ps[:], apow1[:], crow[:], start=False, stop=True)
                nc.vector.tensor_copy(hg[:, 0, :], loc_ps[:])

                # ---------------- per-token scalar math ----------------
                h = hg[:, 0, :]
                nc.vector.tensor_mul(scr1[:], h, d4[:, :, 1])
                nc.vector.scalar_tensor_tensor(scr2[:], scr1[:], 2.0, s0[:],
                                               OP.mult, OP.add)
                nc.vector.tensor_mul(scr1[:], h, h)
                nc.vector.scalar_tensor_tensor(scr1[:], scr1[:], cols3[:, 0:1],
                                               scr2[:], OP.mult, OP.add)
                nc.scalar.activation(scr2[:], scr1[:], AF.Sqrt,
                                     bias=eps_col[:], scale=float(1.0 / D))
                nc.vector.reciprocal(rstd2[:], scr2[:])
                for dcol, ccol, out_q in ((2, 1, 1), (3, 2, 2)):
                    nc.vector.scalar_tensor_tensor(scr1[:], h,
                                                   cols3[:, ccol:ccol + 1],
                                                   d4[:, :, dcol],
                                                   OP.mult, OP.add)
                    nc.vector.tensor_mul(scr2[:], scr1[:], rstd2[:])
                    nc.scalar.activation(hg[:, out_q, :], scr2[:],
                                         AF.Gelu_apprx_tanh)

                # ---------------- phase B: rank-3 + residual ----------------
                for m in range(MAIN_TILES):
                    gi = pre_tiles + m
                    ct_ps = psp.tile([3, 128], F32, tag="misc_ps", name=f"ct_ps{m}", bufs=2)
                    nc.tensor.transpose(ct_ps[:], hg[:, :, gi], ident[:])
                    ct_sb = work.tile([3, 128], F32R, tag="ct_sb", name=f"ct_sb{m}")
                    nc.scalar.copy(ct_sb[:], ct_ps[:])
                    r3_ps = psp.tile([128, D], F32, tag="big_ps", name=f"r3_ps{m}", bufs=2)
                    nc.tensor.matmul(r3_ps[:, 0:512], ct_sb[:], w3r[:, 0:512],
                                     start=True, stop=True)
                    nc.tensor.matmul(r3_ps[:, 512:1024], ct_sb[:], w3r[:, 512:1024],
                                     start=True, stop=True)
                    xt = x_tiles[gi]
                    nc.vector.tensor_add(xt[:], xt[:], r3_ps[:])
                    nc.sync.dma_start(yd[m * 128:(m + 1) * 128, :], xt[:])

            if reps is None:
                body()
            else:
                with tc.For_i(0, reps, 1):
                    body()
    nc.compile()
    return nc


def host_constants(norm_w, W_in, a_log, W_out, ffn_w1, ffn_w2, nt):
    a = 1.0 / (1.0 + np.exp(-np.float64(a_log[0])))
    Wn = (norm_w * W_in[:, 0]).astype(np.float32)
    Wout_row = W_out[0, :].astype(np.float32)
    W10n = (norm_w * ffn_w1[:, 0]).astype(np.float32)
    W11n = (norm_w * ffn_w1[:, 1]).astype(np.float32)
    vw4 = np.stack([Wn, Wout_row, W10n, W11n], axis=1).astype(np.float32)
    w3 = np.stack([Wout_row, ffn_w2[0, :], ffn_w2[1, :]], axis=0).astype(np.float32)
    km = np.arange(128)
    expo = km[None, :] - km[:, None]
    t128 = np.where(expo >= 0, a ** np.maximum(expo, 0), 0.0).astype(np.float32)
    frow = (a ** (127 - km)).astype(np.float32).reshape(128, 1)
    apow1 = (a ** (km + 1)).astype(np.float32).reshape(1, 128)
    alrow = np.full((1, nt), a ** 128, dtype=np.float32)
    cWW = np.float32(Wout_row.astype(np.float64) @ Wout_row.astype(np.float64))
    c0 = np.float32(Wout_row.astype(np.float64) @ W10n.astype(np.float64))
    c1 = np.float32(Wout_row.astype(np.float64) @ W11n.astype(np.float64))
    cols3 = np.tile(np.array([cWW, c0, c1], dtype=np.float32), (128, 1))
    return dict(vw4=vw4, w3=w3, ident=np.eye(128, dtype=np.float32), t128=t128,
                frow=frow, apow1=apow1, alrow=alrow, cols3=cols3), a


def pre_tiles_for(a: float) -> int:
    n = int(np.ceil(np.log(1e-9) / (128 * np.log(a))))
    return min(max(n, 1), 16)


def in_maps_for(x, consts, nt):
    pre = (nt - MAIN_TILES) * 128
    maps = []
    for c in range(N_CORES):
        b, j = c // 2, c % 2
        if j == 0:
            prefix = np.zeros((pre, D), np.float32)
        else:
            prefix = np.ascontiguousarray(x[b, HALF - pre:HALF, :])
        xin = np.concatenate([prefix, x[b, j * HALF:(j + 1) * HALF, :]], axis=0)
        m = {"x_in": np.ascontiguousarray(xin)}
        m.update(consts)
        maps.append(m)
    return maps


def kernel(x, norm_w, W_in, a_log, W_out, ffn_w1, ffn_w2):
    x = np.asarray(x, dtype=np.float32)
    consts, a = host_constants(np.asarray(norm_w), np.asarray(W_in),
                               np.asarray(a_log), np.asarray(W_out),
                               np.asarray(ffn_w1), np.asarray(ffn_w2), nt=34)
    nt = MAIN_TILES + pre_tiles_for(a)
    consts["alrow"] = np.full((1, nt), np.float64(a) ** 128, dtype=np.float32)

    key = ("plain", nt)
    if key not in _cache:
        _cache[key] = build_program(nt)
    nc = _cache[key]

    res = run_bass_kernel_spmd(nc, in_maps_for(x, consts, nt),
                               core_ids=list(range(N_CORES)))
    out = np.empty((B, S, D), np.float32)
    for c in range(N_CORES):
        b, j = c // 2, c % 2
        out[b, j * HALF:(j + 1) * HALF, :] = res.results[c]["y_out"]
    return out


# revision 13
# speedup vs baseline: 3.9848x; 3.9848x over previous
"""Trainium2 Bass kernel for nn_AbsoluteMinimalBlock (rmsnorm -> rank-1 SSM scan -> rmsnorm -> rank-2 FFN).

Math: the whole block is a rank-3 update of x:
    out[t,d] = x[t,d] + h[t]*Wout[d] + g0[t]*W20[d] + g1[t]*W21[d]
  driven by 5 per-token reductions over D:
    d1 = x@(nw*W_in), dW = x@Wout, dA = x@(nw*w1_0), dB = x@(nw*w1_1), S0 = sum(x^2)
  with rstd1 = 1/sqrt(S0/D+eps); u = d1*rstd1; h = scan(a, u);
  D*ms2 = S0 + 2h*dW + h^2*|Wout|^2 (analytic); p_r = (d_r + h*(Wout.W1r))*rstd2;
  g_r = gelu_tanh(p_r * 1).

Sharding: 8 cores = 4 batches x 2 sequence halves; each core prepends a
PRE-tile prefix (zeros for first half / tail of first half for second) so the
scan state is exact to fp32 (a^128 per tile of decay).

Device: token-major tiles [128 tok x 1024]; token t -> (tile t//128, partition
t%128). Reductions over D: PE transposes x into PSUM (8x 128x128 per tile,
fp32 transposes are fast), DVE copies to SBUF as float32r, then f32r matmuls
(lhsT = 4 weight columns per d-slice) accumulate dots [4 x 512] per 4-tile
group; small PE transposes flip them to the per-token [128 x NT] layout.
S0 rides ACT square's accum_out. The scan is 3 matmuls (triangular T128,
a^(127-k) finals row, K=1 carry broadcast) + one 1-partition
tensor_tensor_scan over tile finals. Rank-3 reconstruction: per-tile K=3 f32r
matmuls against W3 [3 x 1024]; DVE adds the PSUM update into the resident x
tile in place; DMA out.
"""
import sys, os
for _p in ("/root/.axon_site/_ro/trn_rl_repo", "/opt/trn_rl_repo"):
    if os.path.isdir(_p) and _p not in sys.path:
        sys.path.append(_p)

import numpy as np
import concourse.bass as bass
import concourse.bacc as bacc
import concourse.mybir as mybir
import concourse.tile as tile
from concourse.bass_utils import run_bass_kernel_spmd

F32 = mybir.dt.float32
F32R = mybir.dt.float32r
BF16 = mybir.dt.bfloat16
AF = mybir.ActivationFunctionType
OP = mybir.AluOpType

N_CORES = 8
B, S, D = 4, 8192, 1024
HALF = S // 2
MAIN_TILES = HALF // 128      # 32
EPS = 1e-6

_cache = {}


def build_program(nt: int, reps=None, internal_io=False, parts=("sq", "tp", "dot", "scan", "b3")):
    parts = set(parts)
    """SPMD program for nt total tiles/core (nt-32 prefix tiles).
    reps: wrap body in For_i for timing probes (None = plain)."""
    pre_tiles = nt - MAIN_TILES
    groups = []
    i = 0
    while i < nt:
        groups.append(list(range(i, min(i + 4, nt))))
        i += 4

    nc = bacc.Bacc("TRN2", target_bir_lowering=False, debug=False, num_devices=N_CORES)

    if internal_io:
        xd = nc.dram_tensor("x_int", [nt * 128, D], F32, kind="Internal").ap()
        yd = nc.dram_tensor("y_int", [HALF, D], F32, kind="Internal").ap()
        dummy_in = nc.dram_tensor("x_in", [128, 4], F32, kind="ExternalInput").ap()
        dummy_out = nc.dram_tensor("y_out", [128, 4], F32, kind="ExternalOutput").ap()
        need_dummy_io = True
    else:
        xd = nc.dram_tensor("x_in", [nt * 128, D], F32, kind="ExternalInput").ap()
        yd = nc.dram_tensor("y_out", [HALF, D], F32, kind="ExternalOutput").ap()
        need_dummy_io = False
    vw4d = nc.dram_tensor("vw4", [D, 4], F32, kind="ExternalInput").ap()
    w3d = nc.dram_tensor("w3", [3, D], F32, kind="ExternalInput").ap()
    identd = nc.dram_tensor("ident", [128, 128], F32, kind="ExternalInput").ap()
    t128d = nc.dram_tensor("t128", [128, 128], F32, kind="ExternalInput").ap()
    frowd = nc.dram_tensor("frow", [128, 1], F32, kind="ExternalInput").ap()
    apow1d = nc.dram_tensor("apow1", [1, 128], F32, kind="ExternalInput").ap()
    alrowd = nc.dram_tensor("alrow", [1, nt], F32, kind="ExternalInput").ap()
    colsd = nc.dram_tensor("cols3", [128, 3], F32, kind="ExternalInput").ap()

    with tile.TileContext(nc) as tc:
        with (
            tc.tile_pool(name="xpool", bufs=1) as xpool,
            tc.tile_pool(name="work", bufs=4) as work,
            tc.tile_pool(name="sq", bufs=3) as sqp,
            tc.tile_pool(name="small", bufs=1) as small,
            tc.tile_pool(name="cst", bufs=1) as cst,
            tc.tile_pool(name="ps", bufs=1, space="PSUM") as psp,
        ):
            # ---- constants ----
            vw4 = cst.tile([128, 8, 4], F32, name="vw4")
            vw4r = cst.tile([128, 8, 4], F32R, name="vw4r")
            w3r = cst.tile([3, D], F32R, name="w3r")
            ident = cst.tile([128, 128], F32, name="ident")
            t128 = cst.tile([128, 128], F32, name="t128")
            frow = cst.tile([128, 1], F32, name="frow")
            apow1 = cst.tile([1, 128], F32, name="apow1")
            alrow = cst.tile([1, nt], F32, name="alrow")
            cols3 = cst.tile([128, 3], F32, name="cols3")
            nc.sync.dma_start(vw4[:], vw4d.rearrange("(k p) q -> p k q", p=128))
            nc.vector.tensor_copy(vw4r[:], vw4[:])
            w3f = cst.tile([3, D], F32, name="w3f")
            nc.sync.dma_start(w3f[:], w3d[:])
            nc.vector.tensor_copy(w3r[:], w3f[:])
            nc.sync.dma_start(ident[:], identd[:])
            nc.sync.dma_start(t128[:], t128d[:])
            nc.sync.dma_start(frow[:], frowd[:])
            nc.sync.dma_start(apow1[:], apow1d[:])
            nc.sync.dma_start(alrow[:], alrowd[:])
            nc.sync.dma_start(cols3[:], colsd[:])

            # ---- per-token arrays ----
            d4 = small.tile([128, nt, 4], F32, name="d4")
            s0 = small.tile([128, nt], F32, name="s0")
            hg = small.tile([128, 3, nt], F32, name="hg")
            u = small.tile([128, nt], F32, name="u")
            scr1 = small.tile([128, nt], F32, name="scr1")
            scr2 = small.tile([128, nt], F32, name="scr2")
            rstd2 = small.tile([128, nt], F32, name="rstd2")
            srow = small.tile([1, nt], F32, name="srow")
            eps_col = small.tile([128, 1], F32, name="eps_col")
            nc.vector.memset(eps_col[:], float(EPS))
            if need_dummy_io:
                dum = small.tile([128, 4], F32, name="dum")
                nc.sync.dma_start(dum[:], dummy_in[:])
                nc.sync.dma_start(dummy_out[:], dum[:])
            crow = small.tile([1, nt], F32, name="crow")

            def body():
                x_tiles = [xpool.tile([128, D], F32, tag=f"x{i}", name=f"x{i}")
                           for i in range(nt)]
                # ---------------- phase A ----------------
                for g in groups:
                    glen = len(g)
                    for i in g:
                        nc.sync.dma_start(x_tiles[i][:], xd[i * 128:(i + 1) * 128, :])
                        if "sq" in parts:
                            sq = sqp.tile([128, D], BF16, tag="sq", name=f"sq{i}")
                            nc.scalar.activation(sq[:], x_tiles[i][:], AF.Square,
                                                 accum_out=s0[:, i:i + 1])
                        elif "sqnoacc" in parts:
                            sq = sqp.tile([128, D], F32, tag="sq", name=f"sq{i}")
                            nc.scalar.activation(sq[:], x_tiles[i][:], AF.Square)
                        elif "sqg" in parts:
                            sq = sqp.tile([128, D], F32, tag="sq", name=f"sq{i}")
                            nc.gpsimd.scalar_tensor_tensor(sq[:], x_tiles[i][:], 1.0,
                                                           x_tiles[i][:], OP.mult,
                                                           OP.mult,
                                                           accum_out=s0[:, i:i + 1])
                    if "tp" not in parts and "tpnc" not in parts:
                        continue
                    dots_ps = psp.tile([4, 512], F32, tag="dots_ps",
                                       name=f"dots_ps{g[0]}", bufs=2)
                    for p in range(4):          # slice pairs (2p, 2p+1)
                        pair_ps = psp.tile([128, D], F32, tag="big_ps",
                                           name=f"pair_ps{g[0]}_{p}", bufs=2)
                        for sl in range(2):
                            k = 2 * p + sl
                            for gi, i in enumerate(g):
                                nc.tensor.transpose(
                                    pair_ps[:, sl * 512 + gi * 128:
                                            sl * 512 + (gi + 1) * 128],
                                    x_tiles[i][:, k * 128:(k + 1) * 128],
                                    ident[:])
                        if "tp" not in parts:
                            continue
                        pair_sb = work.tile([128, D], F32R, tag="pair_sb",
                                            name=f"pair_sb{g[0]}_{p}")
                        # split the PSUM->SBUF move across DVE and ACT
                        w0 = min(512, glen * 128)
                        nc.vector.tensor_copy(pair_sb[:, 0:w0], pair_ps[:, 0:w0])
                        nc.scalar.copy(pair_sb[:, 512:512 + glen * 128],
                                       pair_ps[:, 512:512 + glen * 128])
                        for sl in range(2 if "dot" in parts else 0):
                            k = 2 * p + sl
                            nc.tensor.matmul(dots_ps[:, 0:glen * 128],
                                             vw4r[:, k, :],
                                             pair_sb[:, sl * 512:sl * 512 + glen * 128],
                                             start=(p == 0 and sl == 0),
                                             stop=(p == 3 and sl == 1))
                    if "dot" not in parts:
                        continue
                    dots_sb = work.tile([4, 512], F32, tag="dots_sb",
                                        name=f"dots_sb{g[0]}")
                    nc.vector.tensor_copy(dots_sb[:, 0:glen * 128],
                                          dots_ps[:, 0:glen * 128])
                    for gi, i in enumerate(g):
                        d4t_ps = psp.tile([128, 4], F32, tag="misc_ps",
                                          name=f"d4t_ps{i}", bufs=2)
                        nc.tensor.transpose(d4t_ps[:],
                                            dots_sb[:, gi * 128:(gi + 1) * 128],
                                            ident[0:4, 0:4])
                        nc.scalar.copy(d4[:, i, :], d4t_ps[:])

                # ---------------- scan ----------------
                if "scan" not in parts:
                    return
                nc.scalar.activation(scr1[:], s0[:], AF.Sqrt,
                                     bias=eps_col[:], scale=float(1.0 / D))
                nc.vector.reciprocal(scr2[:], scr1[:])
                nc.vector.tensor_mul(u[:], d4[:, :, 0], scr2[:])

                loc_ps = psp.tile([128, nt], F32, tag="misc_ps", name="loc_ps", bufs=2)
                f_ps = psp.tile([1, nt], F32, tag="misc_ps", name="f_ps", bufs=2)
                nc.tensor.matmul(loc_ps[:], t128[:], u[:], start=True, stop=False)
                nc.tensor.matmul(f_ps[:], frow[:], u[:], start=True, stop=True)
                nc.vector.tensor_tensor_scan(srow[:], alrow[:], f_ps[:], 0.0,
                                             OP.mult, OP.add)
                nc.vector.memset(crow[0:1, 0:1], 0.0)
                nc.vector.tensor_copy(crow[0:1, 1:nt], srow[0:1, 0:nt - 1])
                nc.tensor.matmul(loc_ps[:], apow1[:], crow[:], start=False, stop=True)
                nc.vector.tensor_copy(hg[:, 0, :], loc_ps[:])

                # ---------------- per-token scalar math ----------------
                h = hg[:, 0, :]
                nc.vector.tensor_mul(scr1[:], h, d4[:, :, 1])
                nc.vector.scalar_tensor_tensor(scr2[:], scr1[:], 2.0, s0[:],
                                               OP.mult, OP.add)
                nc.vector.tensor_mul(scr1[:], h, h)
                nc.vector.scalar_tensor_tensor(scr1[:], scr1[:], cols3[:, 0:1],
                                               scr2[:], OP.mult, OP.add)
                nc.scalar.activation(scr2[:], scr1[:], AF.Sqrt,
                                     bias=eps_col[:], scale=float(1.0 / D))
                nc.vector.reciprocal(rstd2[:], scr2[:])
                for dcol, ccol, out_q in ((2, 1, 1), (3, 2, 2)):
                    nc.vector.scalar_tensor_tensor(scr1[:], h,
                                                   cols3[:, ccol:ccol + 1],
                                                   d4[:, :, dcol],
                                                   OP.mult, OP.add)
                    nc.vector.tensor_mul(scr2[:], scr1[:], rstd2[:])
                    nc.scalar.activation(hg[:, out_q, :], scr2[:],
                                         AF.Gelu_apprx_tanh)

                # ---------------- phase B: rank-3 + residual ----------------
                if "b3" not in parts:
                    return
                for m in range(MAIN_TILES):
                    gi = pre_tiles + m
                    ct_ps = psp.tile([3, 128], F32, tag="misc_ps", name=f"ct_ps{m}", bufs=2)
                    nc.tensor.transpose(ct_ps[:], hg[:, :, gi], ident[:])
                    ct_sb = work.tile([3, 128], F32R, tag="ct_sb", name=f"ct_sb{m}")
                    nc.scalar.copy(ct_sb[:], ct_ps[:])
                    r3_ps = psp.tile([128, D], F32, tag="big_ps", name=f"r3_ps{m}", bufs=2)
                    nc.tensor.matmul(r3_ps[:, 0:512], ct_sb[:], w3r[:, 0:512],
                                     start=True, stop=True)
                    nc.tensor.matmul(r3_ps[:, 512:1024], ct_sb[:], w3r[:, 512:1024],
                                     start=True, stop=True)
                    xt = x_tiles[gi]
                    nc.vector.tensor_add(xt[:], xt[:], r3_ps[:])
                    nc.sync.dma_start(yd[m * 128:(m + 1) * 128, :], xt[:])

            if reps is None:
                body()
            else:
                with tc.For_i(0, reps, 1):
                    body()
    nc.compile()
    return nc


def host_constants(norm_w, W_in, a_log, W_out, ffn_w1, ffn_w2, nt):
    a = 1.0 / (1.0 + np.exp(-np.float64(a_log[0])))
    Wn = (norm_w * W_in[:, 0]).astype(np.float32)
    Wout_row = W_out[0, :].astype(np.float32)
    W10n = (norm_w * ffn_w1[:, 0]).astype(np.float32)
    W11n = (norm_w * ffn_w1[:, 1]).astype(np.float32)
    vw4 = np.stack([Wn, Wout_row, W10n, W11n], axis=1).astype(np.float32)
    w3 = np.stack([Wout_row, ffn_w2[0, :], ffn_w2[1, :]], axis=0).astype(np.float32)
    km = np.arange(128)
    expo = km[None, :] - km[:, None]
    t128 = np.where(expo >= 0, a ** np.maximum(expo, 0), 0.0).astype(np.float32)
    frow = (a ** (127 - km)).astype(np.float32).reshape(128, 1)
    apow1 = (a ** (km + 1)).astype(np.float32).reshape(1, 128)
    alrow = np.full((1, nt), a ** 128, dtype=np.float32)
    cWW = np.float32(Wout_row.astype(np.float64) @ Wout_row.astype(np.float64))
    c0 = np.float32(Wout_row.astype(np.float64) @ W10n.astype(np.float64))
    c1 = np.float32(Wout_row.astype(np.float64) @ W11n.astype(np.float64))
    cols3 = np.tile(np.array([cWW, c0, c1], dtype=np.float32), (128, 1))
    return dict(vw4=vw4, w3=w3, ident=np.eye(128, dtype=np.float32), t128=t128,
                frow=frow, apow1=apow1, alrow=alrow, cols3=cols3), a


def pre_tiles_for(a: float) -> int:
    n = int(np.ceil(np.log(1e-9) / (128 * np.log(a))))
    # SBUF keeps all nt tiles resident; cap the prefix (a=sigmoid(a_log) ~ 0.785
    # for the reference inputs -> n=1)
    return min(max(n, 1), 6)


def in_maps_for(x, consts, nt):
    pre = (nt - MAIN_TILES) * 128
    maps = []
    for c in range(N_CORES):
        b, j = c // 2, c % 2
        if j == 0:
            prefix = np.zeros((pre, D), np.float32)
        else:
            prefix = np.ascontiguousarray(x[b, HALF - pre:HALF, :])
        xin = np.concatenate([prefix, x[b, j * HALF:(j + 1) * HALF, :]], axis=0)
        m = {"x_in": np.ascontiguousarray(xin)}
        m.update(consts)
        maps.append(m)
    return maps


def kernel(x, norm_w, W_in, a_log, W_out, ffn_w1, ffn_w2):
    x = np.asarray(x, dtype=np.float32)
    consts, a = host_constants(np.asarray(norm_w), np.asarray(W_in),
                               np.asarray(a_log), np.asarray(W_out),
                               np.asarray(ffn_w1), np.asarray(ffn_w2), nt=34)
    nt = MAIN_TILES + pre_tiles_for(a)
    consts["alrow"] = np.full((1, nt), np.float64(a) ** 128, dtype=np.float32)

    key = ("plain", nt)
    if key not in _cache:
        _cache[key] = build_program(nt)
    nc = _cache[key]

    res = run_bass_kernel_spmd(nc, in_maps_for(x, consts, nt),
                               core_ids=list(range(N_CORES)))
    out = np.empty((B, S, D), np.float32)
    for c in range(N_CORES):
        b, j = c // 2, c % 2
        out[b, j * HALF:(j + 1) * HALF, :] = res.results[c]["y_out"]
    return out
